# revision 33
# baseline (speedup 1.0000x reference)
"""Trainium2 Bass kernel for the differentiable gaussian-splat renderer.

Full-input contract: kernel(**inputs) takes the unsharded inputs and returns
the full [2*16, 3, 32, 32] output.

Math (per pose):
    cam = positions @ R.T + t ;  pj = (fx*cam_x/cam_z + cx, fy*cam_y/cam_z + cy)
    w[n, p] = op_n * exp(-0.5*((px-ax_n)^2 + (py-ay_n)^2)/s_n^2)
    img = (w.T @ (colors)) / (w.T @ 1 + 1e-8)

The gaussian weight is separable: w = op * wx[n,px] * wy[n,py], so instead of
N*HW exponentials we need N*(W + H) and the pixel accumulation becomes a
K=128-chunked matmul  out[py, (c,px)] += wy_chunk.T @ (ca_chunk ⊙ wx_chunk).

Sharding: 8 independent cores = 2 poses x 4 px-column blocks (32 px each).
No collectives; each core computes all 4096 gaussians for its (pose, px-block)
and writes a [128, 96] slab = (py, 32c+px_local). Host reassembles.

Per-gaussian screen coords are centered (ax' = ax-64, px' = px-64) and the
exp argument is evaluated as a K=3 matmul of per-gaussian quadratic
coefficients [g, -2*g*ax', g*ax'^2] against a block-diagonal pixel basis
[q'^2, q', 1] (4 chunks of 128 py per 512-wide matmul; 16 chunks of 32 px).
Numerics validated: L2 rel err ~2e-5 vs the fp32 jax reference.
"""

import numpy as np

H = 128
W = 128
FX = 120.0
FY = 120.0
CX = 64.0
CY = 64.0
N = 4096
NCHUNK = 32          # 4096 / 128
NPOSE = 2
PXB = 32             # px columns per core
NBLK = 4             # px blocks
F32 = np.float32

_CACHE = {}


def _quat2mat(q):
    q = np.asarray(q, dtype=np.float64)
    q = q / np.linalg.norm(q)
    w, x, y, z = q
    return np.array([
        [1 - 2 * (y * y + z * z), 2 * (x * y - z * w), 2 * (x * z + y * w)],
        [2 * (x * y + z * w), 1 - 2 * (x * x + z * z), 2 * (y * z - x * w)],
        [2 * (x * z - y * w), 2 * (y * z + x * w), 1 - 2 * (x * x + y * y)],
    ])


def _build_program():
    """Build the SPMD Bass/Tile program (same program on every core)."""
    import concourse.bass as bass
    import concourse.bacc as bacc
    import concourse.tile as tile
    import concourse.mybir as mybir
    from contextlib import ExitStack

    dt = mybir.dt.float32
    nc = bacc.Bacc()

    # ---- DRAM I/O (per-core shapes) ----
    # inp128 cols: 0:128 pos4 | 128:256 colors1 | 256:288 opac |
    #              288:416 ident | 416:428 rb (pre-broadcast)
    inp128_d = nc.dram_tensor("inp128", [128, 428], dt, kind="ExternalInput").ap()
    # inp48 cols: 0:2048 basis_y (4 block-diag variants) | 2048:2560 basis_x
    inp48_d = nc.dram_tensor("inp48", [48, 2560], dt, kind="ExternalInput").ap()
    out_d = nc.dram_tensor("out", [128, 96], dt, kind="ExternalOutput").ap()

    mult = mybir.AluOpType.mult
    add = mybir.AluOpType.add
    EXP = mybir.ActivationFunctionType.Exp

    with tile.TileContext(nc) as tc, ExitStack() as ctx:
        const = ctx.enter_context(tc.tile_pool(name="const", bufs=1))
        work = ctx.enter_context(tc.tile_pool(name="work", bufs=1))
        xpool = ctx.enter_context(tc.tile_pool(name="xpool", bufs=8))
        psum_arg = ctx.enter_context(tc.tile_pool(name="psum_arg", bufs=2, space="PSUM"))
        psum_tp = ctx.enter_context(tc.tile_pool(name="psum_tp", bufs=1, space="PSUM"))
        psum_out = ctx.enter_context(tc.tile_pool(name="psum_out", bufs=1, space="PSUM"))

        # ---- load inputs (one DMA per SBUF tile: consumers then wait on a
        #      single DMA semaphore lane each) ----
        po = psum_out.tile([128, 128], dt, tag="po")  # claim psum bank 0 first
        inp128 = const.tile([128, 428], dt, tag="inp128")
        nc.sync.dma_start(out=inp128[:], in_=inp128_d)
        inp48 = const.tile([48, 2560], dt, tag="inp48")
        nc.sync.dma_start(out=inp48[:], in_=inp48_d)

        colors1 = inp128[:, 128:256]
        opac = inp128[:, 256:288]
        ident = inp128[:, 288:416]
        rb = inp128[:, 416:428]
        basis_y = inp48[:, 0:2048]
        basis_x = inp48[:, 2048:2560]

        xg = inp128[:, 0:32]
        yg = inp128[:, 32:64]
        zg = inp128[:, 64:96]
        sg = inp128[:, 96:128]

        # ---- projection: u,v,zc = A @ [x,y,z,1]  (DVE, [128,32] ops) ----
        def lin3(c0):
            acc = work.tile([128, 32], dt, tag=f"acc{c0}")
            nc.vector.tensor_scalar(out=acc[:], in0=xg, scalar1=rb[:, c0:c0 + 1],
                                    scalar2=rb[:, c0 + 3:c0 + 4], op0=mult, op1=add)
            t1 = work.tile([128, 32], dt, tag=f"t1{c0}")
            nc.vector.tensor_scalar(out=t1[:], in0=yg, scalar1=rb[:, c0 + 1:c0 + 2],
                                    scalar2=None, op0=mult)
            nc.vector.tensor_add(out=acc[:], in0=acc[:], in1=t1[:])
            t2 = work.tile([128, 32], dt, tag=f"t2{c0}")
            nc.vector.tensor_scalar(out=t2[:], in0=zg, scalar1=rb[:, c0 + 2:c0 + 3],
                                    scalar2=None, op0=mult)
            nc.vector.tensor_add(out=acc[:], in0=acc[:], in1=t2[:])
            return acc

        u = lin3(0)
        v = lin3(4)
        zc = lin3(8)
        zr = work.tile([128, 32], dt, tag="zr")
        nc.vector.reciprocal(out=zr[:], in_=zc[:])
        ax = work.tile([128, 32], dt, tag="ax")
        nc.vector.tensor_mul(out=ax[:], in0=u[:], in1=zr[:])
        ay = work.tile([128, 32], dt, tag="ay")
        nc.vector.tensor_mul(out=ay[:], in0=v[:], in1=zr[:])

        s2 = work.tile([128, 32], dt, tag="s2")
        nc.vector.tensor_mul(out=s2[:], in0=sg, in1=sg)
        gr = work.tile([128, 32], dt, tag="gr")
        nc.vector.reciprocal(out=gr[:], in_=s2[:])
        g = work.tile([128, 32], dt, tag="g")
        nc.vector.tensor_scalar(out=g[:], in0=gr[:], scalar1=-0.5, scalar2=None,
                                op0=mult)

        # ---- coef packs [128, 96], col 3j+q = (A=g, B=-2g*ax', C=g*ax'^2) ----
        def coef_pack(axy, name):
            pk = work.tile([128, 96], dt, tag=f"pack{name}")
            pkr = pk[:].rearrange("p (j q) -> p q j", q=3)
            ga = work.tile([128, 32], dt, tag=f"ga{name}")
            nc.vector.tensor_mul(out=ga[:], in0=g[:], in1=axy[:])
            nc.vector.tensor_copy(out=pkr[:, 0, :], in_=g[:])
            nc.vector.tensor_scalar(out=pkr[:, 1, :], in0=ga[:], scalar1=-2.0,
                                    scalar2=None, op0=mult)
            nc.vector.tensor_mul(out=pkr[:, 2, :], in0=ga[:], in1=axy[:])
            return pk

        pack_x = coef_pack(ax, "x")
        pack_y = coef_pack(ay, "y")

        # ---- transpose packs -> two [48, 128] coefT tiles each (PE needs
        #      lhsT at base partition 0, so transpose each 48-col half) ----
        def transpose_pack(pk, name):
            cts = []
            for h in range(2):
                tp = psum_tp.tile([48, 128], dt, tag="tp")
                nc.tensor.transpose(tp[:], pk[:, 48 * h:48 * h + 48], ident)
                ct = const.tile([48, 128], dt, tag=f"coefT{name}{h}")
                nc.vector.tensor_copy(out=ct[:], in_=tp[:])
                cts.append(ct)
            return cts

        coefT_x = transpose_pack(pack_x, "x")
        coefT_y = transpose_pack(pack_y, "y")

        # ---- ca = colors1 * opac  [128, 128] (cols 32c+j) ----
        ca = const.tile([128, 128], dt, tag="ca")
        ca_r = ca[:].rearrange("p (c j) -> p c j", c=4)
        col_r = colors1.rearrange("p (c j) -> p c j", c=4)
        op_b = opac.unsqueeze(1).broadcast_to([128, 4, 32])
        nc.vector.tensor_mul(out=ca_r, in0=col_r, in1=op_b)

        # ---- wx: 2 matmuls [48,128]x[48,512] -> one exp -> w_x [128,1024] ----
        pa = psum_arg.tile([128, 1024], dt, tag="pa")
        for gx in range(2):
            nc.tensor.matmul(pa[:, 512 * gx:512 * gx + 512], lhsT=coefT_x[gx][:],
                             rhs=basis_x, start=True, stop=True)
        w_x = const.tile([128, 1024], dt, tag="wx")
        nc.scalar.activation(out=w_x[:], in_=pa[:], func=EXP)

        # ---- wy: 8 matmuls [48,128]x[48,512] (block-diag basis variant per
        #      4-chunk group) in rounds of 2 -> exp [128,1024] ----
        w_y = []
        for rnd in range(4):
            pa = psum_arg.tile([128, 1024], dt, tag="pa")
            for gg in range(2):
                grp = rnd * 2 + gg          # 4-chunk group 0..7
                half, s = grp // 4, grp % 4
                nc.tensor.matmul(pa[:, 512 * gg:512 * gg + 512],
                                 lhsT=coefT_y[half][:],
                                 rhs=basis_y[:, 512 * s:512 * s + 512],
                                 start=True, stop=True)
            wt = const.tile([128, 1024], dt, tag=f"wy{rnd}")
            nc.scalar.activation(out=wt[:], in_=pa[:], func=EXP)
            w_y.append(wt)

        # ---- main accumulation: out[py, (c,px)] += wy_j.T @ X_j ----
        for grp in range(8):           # 4 chunks per group
            Xg = xpool.tile([128, 512], dt, tag="X")
            Xg_r = Xg[:].rearrange("p (j c x) -> p j c x", j=4, c=4)
            j0 = grp * 4
            # ca cols 32c+j -> dims (j,c,px): slice j0..j0+4
            ca_in = ca[:].rearrange("p (c j) -> p j c", c=4)[:, j0:j0 + 4, :]
            ca_in = ca_in.unsqueeze(3).broadcast_to([128, 4, 4, 32])
            # wx chunk j at col 512*(j//16) + 32*(j%16)
            o0 = 512 * (j0 // 16) + (j0 % 16) * 32
            wx_in = w_x[:, o0:o0 + 128].rearrange("p (j x) -> p j x", j=4)
            wx_in = wx_in.unsqueeze(2).broadcast_to([128, 4, 4, 32])
            nc.vector.tensor_mul(out=Xg_r, in0=ca_in, in1=wx_in)
            wyt = w_y[grp // 2]
            for jj in range(4):
                j = j0 + jj
                oy = (j % 8) * 128
                nc.tensor.matmul(po[:], lhsT=wyt[:, oy:oy + 128],
                                 rhs=Xg[:, 128 * jj:128 * jj + 128],
                                 start=(j == 0), stop=(j == NCHUNK - 1))

        # ---- normalize: img = num * (1/(den + 1e-8)) ----
        dent = work.tile([128, 32], dt, tag="dent")
        nc.vector.tensor_scalar(out=dent[:], in0=po[:, 96:128], scalar1=1e-8,
                                scalar2=None, op0=add)
        dr = work.tile([128, 32], dt, tag="dr")
        nc.vector.reciprocal(out=dr[:], in_=dent[:])
        img = work.tile([128, 96], dt, tag="img")
        img_r = img[:].rearrange("p (c x) -> p c x", c=3)
        num_r = po[:, 0:96].rearrange("p (c x) -> p c x", c=3)
        dr_b = dr[:].unsqueeze(1).broadcast_to([128, 3, 32])
        nc.vector.tensor_mul(out=img_r, in0=num_r, in1=dr_b)
        nc.sync.dma_start(out=out_d, in_=img[:])

    nc.compile()   # legalizes sync waits (HW allows 1/instruction) etc.
    return nc


def _host_prep(positions, colors, opacities, scales, qvec, tvec):
    """Build the 8 per-core input maps."""
    positions = np.ascontiguousarray(np.asarray(positions, dtype=F32))
    colors = np.ascontiguousarray(np.asarray(colors, dtype=F32))
    opacities = np.ascontiguousarray(np.asarray(opacities, dtype=F32))
    scales = np.ascontiguousarray(np.asarray(scales, dtype=F32))
    qvec = np.asarray(qvec, dtype=F32)
    tvec = np.asarray(tvec, dtype=F32)

    pos_v = positions.reshape(NCHUNK, 128, 3)
    sc_v = scales.reshape(NCHUNK, 128)
    pos4 = np.concatenate([pos_v[:, :, 0].T, pos_v[:, :, 1].T, pos_v[:, :, 2].T,
                           sc_v.T], axis=1).astype(F32)          # [128, 128]
    col_v = colors.reshape(NCHUNK, 128, 3)
    colors1 = np.concatenate([col_v[:, :, 0].T, col_v[:, :, 1].T, col_v[:, :, 2].T,
                              np.ones((128, NCHUNK), F32)], axis=1).astype(F32)
    opac = opacities.reshape(NCHUNK, 128).T.astype(F32)          # [128, 32]

    # folded pose matrices (centering: cx-64 = cy-64 = 0)
    rbs = []
    for p in range(NPOSE):
        R = _quat2mat(qvec[p])
        t = tvec[p].astype(np.float64)
        A = np.zeros((3, 4))
        A[0, :3] = FX * R[0] + (CX - 64.0) * R[2]
        A[0, 3] = FX * t[0] + (CX - 64.0) * t[2]
        A[1, :3] = FY * R[1] + (CY - 64.0) * R[2]
        A[1, 3] = FY * t[1] + (CY - 64.0) * t[2]
        A[2, :3] = R[2]
        A[2, 3] = t[2]
        rbs.append(A.reshape(1, 12).astype(F32))

    # basis_y[:, 512s:512s+512] is the block-diag basis for 4-chunk group s:
    # row 12*c'+q nonzero (= [py'^2, py', 1][q]) only in cols 128*(c'-4s)+py
    py = (np.arange(128) - 64).astype(F32)
    basis_y = np.zeros((48, 2048), F32)
    for s in range(4):
        for ci in range(4):
            c = 4 * s + ci
            sl = slice(512 * s + 128 * ci, 512 * s + 128 * ci + 128)
            basis_y[3 * c + 0, sl] = py * py
            basis_y[3 * c + 1, sl] = py
            basis_y[3 * c + 2, sl] = 1.0

    basis_xs = []
    for b in range(NBLK):
        px = (np.arange(PXB * b, PXB * b + PXB) - 64).astype(F32)
        bx = np.zeros((48, 512), F32)
        for c in range(16):
            bx[3 * c + 0, 32 * c:32 * c + 32] = px * px
            bx[3 * c + 1, 32 * c:32 * c + 32] = px
            bx[3 * c + 2, 32 * c:32 * c + 32] = 1.0
        basis_xs.append(bx)

    ident = np.eye(128, dtype=F32)

    in_maps = []
    for core in range(8):
        p, b = core // NBLK, core % NBLK
        inp128 = np.concatenate(
            [pos4, colors1, opac, ident, np.tile(rbs[p], (128, 1))],
            axis=1).astype(F32)                       # [128, 428]
        inp48 = np.concatenate([basis_y, basis_xs[b]], axis=1).astype(F32)
        in_maps.append({"inp128": inp128, "inp48": inp48})
    return in_maps


def _assemble(slabs):
    """slabs: list of 8 [128, 96] arrays -> [NPOSE*16, 3, 32, 32] output."""
    out = []
    for p in range(NPOSE):
        img = np.zeros((H, W, 3), F32)
        for b in range(NBLK):
            slab = slabs[p * NBLK + b]
            for c in range(3):
                img[:, PXB * b:PXB * b + PXB, c] = slab[:, 32 * c:32 * c + 32]
        tiles = img.reshape(H * W, 3).reshape(16, 1024, 3)
        tiles = tiles.transpose(0, 2, 1).reshape(16, 3, 32, 32)
        out.append(tiles)
    return np.concatenate(out, axis=0).astype(F32)


def kernel(positions, colors, opacities, scales, qvec, tvec, _trace=False):
    from concourse.bass_utils import run_bass_kernel_spmd

    if "nc" not in _CACHE:
        _CACHE["nc"] = _build_program()
    nc = _CACHE["nc"]

    in_maps = _host_prep(positions, colors, opacities, scales, qvec, tvec)
    res = run_bass_kernel_spmd(nc, in_maps, core_ids=list(range(8)),
                               trace=_trace)
    slabs = [np.asarray(res.results[c]["out"]) for c in range(8)]
    out = _assemble(slabs)
    if _trace:
        _CACHE["last_result"] = res
    return out


# revision 45
# speedup vs baseline: 1.2972x; 1.2972x over previous
"""Trainium2 Bass kernel for the differentiable gaussian-splat renderer.

Full-input contract: kernel(**inputs) takes the unsharded inputs and returns
the full [2*16, 3, 32, 32] output.

Math (per pose):
    cam = positions @ R.T + t ;  pj = (fx*cam_x/cam_z + cx, fy*cam_y/cam_z + cy)
    w[n, p] = op_n * exp(-0.5*((px-ax_n)^2 + (py-ay_n)^2)/s_n^2)
    img = (w.T @ colors) / (w.T @ 1 + 1e-8)

The gaussian weight is separable: w = op * wx[n,px] * wy[n,py], so instead of
N*HW exponentials we need N*(W + H) and the pixel accumulation becomes a
K=128-chunked matmul  out[py, (c,px)] += wy_chunk.T @ (ca_chunk (*) wx_chunk).

Sharding: 8 independent cores = 2 poses x 4 px-column blocks (32 px each).
No collectives; each core computes all 4096 gaussians for its (pose, px-block)
and writes a [128, 96] slab = (py, 32c+px_local). Host reassembles.

The exp argument g*(q'-ax')^2 (q' centered at 64) is evaluated as a matmul of
per-gaussian quadratic coefficients [g, -2*g*ax', g*ax'^2] against a
block-diagonal pixel basis [q'^2, q', 1]. For PE speed the coefficients are
split into 3 bf16 pieces each (exact to ~24 bits; pixel basis rows are
bf16-exact integers, q'^2 split into hi+lo rows), so the arg matmuls run at
1 cycle/column instead of fp32's 4. The main accumulation matmul runs on
fp32r (positive, well-conditioned sums).
"""

import numpy as np

H = 128
W = 128
FX = 120.0
FY = 120.0
CX = 64.0
CY = 64.0
N = 4096
NCHUNK = 32          # 4096 / 128
NPOSE = 2
PXB = 32             # px columns per core
NBLK = 4             # px blocks
F32 = np.float32

# main-matmul operand dtype: "fp32r" or "fp32"
MAIN_DTYPE = "fp32r"

_CACHE = {}


def _quat2mat(q):
    q = np.asarray(q, dtype=np.float64)
    q = q / np.linalg.norm(q)
    w, x, y, z = q
    return np.array([
        [1 - 2 * (y * y + z * z), 2 * (x * y - z * w), 2 * (x * z + y * w)],
        [2 * (x * y + z * w), 1 - 2 * (x * x + z * z), 2 * (y * z - x * w)],
        [2 * (x * z - y * w), 2 * (y * z + x * w), 1 - 2 * (x * x + y * y)],
    ])


def _build_program():
    """Build the SPMD Bass/Tile program (same program on every core)."""
    import concourse.bacc as bacc
    import concourse.tile as tile
    import concourse.mybir as mybir
    from contextlib import ExitStack

    dt = mybir.dt.float32
    bf = mybir.dt.bfloat16
    dtm = mybir.dt.float32r if MAIN_DTYPE == "fp32r" else mybir.dt.float32
    nc = bacc.Bacc()

    # ---- DRAM I/O (per-core shapes) ----
    # inp128 cols: 0:128 pos4 | 128:256 colors1 | 256:288 opac |
    #              288:416 ident | 416:428 rb (pre-broadcast)
    inp128_d = nc.dram_tensor("inp128", [128, 428], dt, kind="ExternalInput").ap()
    # bas (bf16): rows 0:64 cols 0:512 = basis_y ; cols 512:768 = basis_x ;
    #             cols 768:896 = bf16 identity (for PE transposes)
    bas_d = nc.dram_tensor("bas", [128, 896], bf, kind="ExternalInput").ap()
    out_d = nc.dram_tensor("out", [128, 96], dt, kind="ExternalOutput").ap()

    mult = mybir.AluOpType.mult
    add = mybir.AluOpType.add
    sub = mybir.AluOpType.subtract
    EXP = mybir.ActivationFunctionType.Exp

    with tile.TileContext(nc) as tc, ExitStack() as ctx:
        const = ctx.enter_context(tc.tile_pool(name="const", bufs=1))
        work = ctx.enter_context(tc.tile_pool(name="work", bufs=1))
        xpool = ctx.enter_context(tc.tile_pool(name="xpool", bufs=8))
        psum_arg = ctx.enter_context(tc.tile_pool(name="psum_arg", bufs=2, space="PSUM"))
        psum_tp = ctx.enter_context(tc.tile_pool(name="psum_tp", bufs=2, space="PSUM"))
        psum_out = ctx.enter_context(tc.tile_pool(name="psum_out", bufs=1, space="PSUM"))

        po = psum_out.tile([128, 128], dt, tag="po")  # claim psum bank 0 first
        inp128 = const.tile([128, 428], dt, tag="inp128")
        nc.sync.dma_start(out=inp128[:], in_=inp128_d)
        bas = const.tile([128, 896], bf, tag="bas")
        nc.sync.dma_start(out=bas[:], in_=bas_d)
        ident_bf = bas[:, 768:896]

        colors1 = inp128[:, 128:256]
        opac = inp128[:, 256:288]
        ident = inp128[:, 288:416]
        rb = inp128[:, 416:428]
        # basis_y duplicated at rows 0:64 and 64:128 (matmul requires lhsT and
        # rhs to share a base partition; odd groups' lhsT sits at rows 64:128)
        basis_y2 = [bas[0:64, 0:512], bas[64:128, 0:512]]
        basis_x = bas[:, 512:768]

        xg = inp128[:, 0:32]
        yg = inp128[:, 32:64]
        zg = inp128[:, 64:96]
        sg = inp128[:, 96:128]

        # ---- projection: u,v,zc = A @ [x,y,z,1]; u-chain on DVE, v-chain on
        #      GpSimd so the serial chains run concurrently ----
        def lin3(eng, c0):
            acc = work.tile([128, 32], dt, tag=f"acc{c0}")
            eng.tensor_scalar(out=acc[:], in0=xg, scalar1=rb[:, c0:c0 + 1],
                              scalar2=rb[:, c0 + 3:c0 + 4], op0=mult, op1=add)
            t1 = work.tile([128, 32], dt, tag=f"t1{c0}")
            eng.tensor_scalar(out=t1[:], in0=yg, scalar1=rb[:, c0 + 1:c0 + 2],
                              scalar2=None, op0=mult)
            eng.tensor_add(out=acc[:], in0=acc[:], in1=t1[:])
            t2 = work.tile([128, 32], dt, tag=f"t2{c0}")
            eng.tensor_scalar(out=t2[:], in0=zg, scalar1=rb[:, c0 + 2:c0 + 3],
                              scalar2=None, op0=mult)
            eng.tensor_add(out=acc[:], in0=acc[:], in1=t2[:])
            return acc

        u = lin3(nc.vector, 0)
        v = lin3(nc.gpsimd, 4)
        zc = lin3(nc.vector, 8)
        zr = work.tile([128, 32], dt, tag="zr")
        nc.vector.reciprocal(out=zr[:], in_=zc[:])
        ax = work.tile([128, 32], dt, tag="ax")
        nc.vector.tensor_mul(out=ax[:], in0=u[:], in1=zr[:])
        ay = work.tile([128, 32], dt, tag="ay")
        nc.gpsimd.tensor_mul(out=ay[:], in0=v[:], in1=zr[:])

        s2 = work.tile([128, 32], dt, tag="s2")
        nc.gpsimd.tensor_mul(out=s2[:], in0=sg, in1=sg)
        gr = work.tile([128, 32], dt, tag="gr")
        nc.vector.reciprocal(out=gr[:], in_=s2[:])
        g = work.tile([128, 32], dt, tag="g")
        nc.vector.tensor_scalar(out=g[:], in0=gr[:], scalar1=-0.5, scalar2=None,
                                op0=mult)

        # ---- bf16 coef packs [128, 512], col 16*j + r; rows per chunk:
        #      (A1,A1,A2,A2,A3,A3,B1,B2,B3,C1,C2,C3,0,0,0,0) ----
        pack_x = const.tile([128, 512], bf, tag="packx")
        pack_y = const.tile([128, 512], bf, tag="packy")
        nc.gpsimd.memset(pack_x[:], 0.0)
        nc.gpsimd.memset(pack_y[:], 0.0)

        def prow(pk, r):
            # strided view: row r of each chunk -> [128, 32]
            return pk[:].rearrange("p (j r) -> p r j", r=16)[:, r, :]

        def split3(eng, src, pk, r0, name):
            """Write bf16 pieces of src to pack rows r0, r0+1, r0+2."""
            eng.tensor_copy(out=prow(pk, r0), in_=src[:])
            res1 = work.tile([128, 32], dt, tag=f"res1{name}")
            eng.tensor_tensor(out=res1[:], in0=src[:], in1=prow(pk, r0), op=sub)
            eng.tensor_copy(out=prow(pk, r0 + 1), in_=res1[:])
            res2 = work.tile([128, 32], dt, tag=f"res2{name}")
            eng.tensor_tensor(out=res2[:], in0=res1[:], in1=prow(pk, r0 + 1), op=sub)
            eng.tensor_copy(out=prow(pk, r0 + 2), in_=res2[:])

        # A pieces land in rows 0,1,2; remap to (A1,A1,A2,A2,A3,A3) and copy
        # the block to pack_y (A = g is shared between the axes).
        split3(nc.vector, g, pack_x, 0, "A")

        pxr_x = pack_x[:].rearrange("p (j r) -> p r j", r=16)
        pxr_y = pack_y[:].rearrange("p (j r) -> p r j", r=16)
        # duplicate A rows: rows written by split3 were 0,1,2 — remap:
        # shift row1->row2... simpler: rewrite A pieces at correct rows now.
        # (split3 wrote rows 0,1,2; we need them at 0,2,4 with dups at 1,3,5)
        nc.gpsimd.tensor_copy(out=pxr_x[:, 4, :], in_=pxr_x[:, 2, :])
        nc.gpsimd.tensor_copy(out=pxr_x[:, 2, :], in_=pxr_x[:, 1, :])
        nc.gpsimd.tensor_copy(out=pxr_x[:, 1, :], in_=pxr_x[:, 0, :])
        nc.gpsimd.tensor_copy(out=pxr_x[:, 3, :], in_=pxr_x[:, 2, :])
        nc.gpsimd.tensor_copy(out=pxr_x[:, 5, :], in_=pxr_x[:, 4, :])
        # copy A block (rows 0..5) to pack_y
        nc.gpsimd.tensor_copy(out=pxr_y[:, 0:6, :], in_=pxr_x[:, 0:6, :])

        def bc_coefs(eng, axy, pk, name):
            ga = work.tile([128, 32], dt, tag=f"ga{name}")
            eng.tensor_mul(out=ga[:], in0=g[:], in1=axy[:])
            B = work.tile([128, 32], dt, tag=f"B{name}")
            eng.tensor_scalar(out=B[:], in0=ga[:], scalar1=-2.0, scalar2=None,
                              op0=mult)
            C = work.tile([128, 32], dt, tag=f"C{name}")
            eng.tensor_mul(out=C[:], in0=ga[:], in1=axy[:])
            split3(eng, B, pk, 6, f"B{name}")
            split3(eng, C, pk, 9, f"C{name}")

        bc_coefs(nc.vector, ax, pack_x, "x")
        bc_coefs(nc.gpsimd, ay, pack_y, "y")

        # ---- transpose packs: 4 x [128,128] per axis -> coefT bf16 tiles ----
        def transpose_pack(pk, name):
            cts = []
            for t in range(4):
                tp = psum_tp.tile([128, 128], bf, tag="tp")
                nc.tensor.transpose(tp[:], pk[:, 128 * t:128 * t + 128], ident_bf)
                ct = const.tile([128, 128], bf, tag=f"coefT{name}{t}")
                nc.vector.tensor_copy(out=ct[:], in_=tp[:])
                cts.append(ct)
            return cts

        coefT_x = transpose_pack(pack_x, "x")
        coefT_y = transpose_pack(pack_y, "y")

        # ---- ca = colors1 * opac  [128, 128] (cols 32c+j) ----
        ca = const.tile([128, 128], dt, tag="ca")
        ca_r = ca[:].rearrange("p (c j) -> p c j", c=4)
        col_r = colors1.rearrange("p (c j) -> p c j", c=4)
        op_b = opac.unsqueeze(1).broadcast_to([128, 4, 32])
        nc.vector.tensor_mul(out=ca_r, in0=col_r, in1=op_b)

        # ---- wx args: 4 matmuls [128,128]x[128,256] -> one exp ----
        pa = psum_arg.tile([128, 1024], dt, tag="pa")
        for t in range(4):
            nc.tensor.matmul(pa[:, 256 * t:256 * t + 256], lhsT=coefT_x[t][:],
                             rhs=basis_x, start=True, stop=True)
        w_x = const.tile([128, 1024], dt, tag="wx")
        nc.scalar.activation(out=w_x[:], in_=pa[:], func=EXP)

        # ---- wy args: 8 matmuls [64,128]x[64,512] in rounds of 2 -> exp ----
        w_y = []
        for rnd in range(4):
            pa = psum_arg.tile([128, 1024], dt, tag="pa")
            for gg in range(2):
                grp = rnd * 2 + gg          # 4-chunk group 0..7
                ctile = coefT_y[grp // 2]
                r0 = 64 * (grp % 2)
                nc.tensor.matmul(pa[:, 512 * gg:512 * gg + 512],
                                 lhsT=ctile[r0:r0 + 64, :],
                                 rhs=basis_y2[grp % 2], start=True, stop=True)
            wt = const.tile([128, 1024], dtm, tag=f"wy{rnd}")
            nc.scalar.activation(out=wt[:], in_=pa[:], func=EXP)
            w_y.append(wt)

        # ---- main accumulation: out[py, (c,px)] += wy_j.T @ X_j ----
        for grp in range(8):           # 4 chunks per group
            Xg = xpool.tile([128, 512], dtm, tag="X")
            Xg_r = Xg[:].rearrange("p (j c x) -> p j c x", j=4, c=4)
            j0 = grp * 4
            ca_in = ca[:].rearrange("p (c j) -> p j c", c=4)[:, j0:j0 + 4, :]
            ca_in = ca_in.unsqueeze(3).broadcast_to([128, 4, 4, 32])
            # wx chunk j at col 32*j
            wx_in = w_x[:, 32 * j0:32 * j0 + 128].rearrange(
                "p (j x) -> p j x", j=4)
            wx_in = wx_in.unsqueeze(2).broadcast_to([128, 4, 4, 32])
            eng = nc.gpsimd if grp in (5, 6, 7) else nc.vector
            eng.tensor_mul(out=Xg_r, in0=ca_in, in1=wx_in)
            wyt = w_y[grp // 2]
            for jj in range(4):
                j = j0 + jj
                oy = (j % 8) * 128
                nc.tensor.matmul(po[:], lhsT=wyt[:, oy:oy + 128],
                                 rhs=Xg[:, 128 * jj:128 * jj + 128],
                                 start=(j == 0), stop=(j == NCHUNK - 1))

        # ---- normalize: img = num * (1/(den + 1e-8)) ----
        dent = work.tile([128, 32], dt, tag="dent")
        nc.vector.tensor_scalar(out=dent[:], in0=po[:, 96:128], scalar1=1e-8,
                                scalar2=None, op0=add)
        dr = work.tile([128, 32], dt, tag="dr")
        nc.vector.reciprocal(out=dr[:], in_=dent[:])
        img = work.tile([128, 96], dt, tag="img")
        img_r = img[:].rearrange("p (c x) -> p c x", c=3)
        num_r = po[:, 0:96].rearrange("p (c x) -> p c x", c=3)
        dr_b = dr[:].unsqueeze(1).broadcast_to([128, 3, 32])
        nc.vector.tensor_mul(out=img_r, in0=num_r, in1=dr_b)
        nc.sync.dma_start(out=out_d, in_=img[:])

    nc.compile()   # legalizes sync waits (HW allows 1/instruction) etc.
    return nc


def _host_prep(positions, colors, opacities, scales, qvec, tvec):
    """Build the 8 per-core input maps."""
    import ml_dtypes
    bf = ml_dtypes.bfloat16

    positions = np.ascontiguousarray(np.asarray(positions, dtype=F32))
    colors = np.ascontiguousarray(np.asarray(colors, dtype=F32))
    opacities = np.ascontiguousarray(np.asarray(opacities, dtype=F32))
    scales = np.ascontiguousarray(np.asarray(scales, dtype=F32))
    qvec = np.asarray(qvec, dtype=F32)
    tvec = np.asarray(tvec, dtype=F32)

    pos_v = positions.reshape(NCHUNK, 128, 3)
    sc_v = scales.reshape(NCHUNK, 128)
    pos4 = np.concatenate([pos_v[:, :, 0].T, pos_v[:, :, 1].T, pos_v[:, :, 2].T,
                           sc_v.T], axis=1).astype(F32)          # [128, 128]
    col_v = colors.reshape(NCHUNK, 128, 3)
    colors1 = np.concatenate([col_v[:, :, 0].T, col_v[:, :, 1].T, col_v[:, :, 2].T,
                              np.ones((128, NCHUNK), F32)], axis=1).astype(F32)
    opac = opacities.reshape(NCHUNK, 128).T.astype(F32)          # [128, 32]

    # folded pose matrices (centering: cx-64 = cy-64 = 0)
    rbs = []
    for p in range(NPOSE):
        R = _quat2mat(qvec[p])
        t = tvec[p].astype(np.float64)
        A = np.zeros((3, 4))
        A[0, :3] = FX * R[0] + (CX - 64.0) * R[2]
        A[0, 3] = FX * t[0] + (CX - 64.0) * t[2]
        A[1, :3] = FY * R[1] + (CY - 64.0) * R[2]
        A[1, 3] = FY * t[1] + (CY - 64.0) * t[2]
        A[2, :3] = R[2]
        A[2, 3] = t[2]
        rbs.append(A.reshape(1, 12).astype(F32))

    def basis_rows(q):
        """[16, len(q)] bf16 rows: p2h,p2l,p2h,p2l,p2h,p2l,q,q,q,1,1,1,0*4."""
        q = q.astype(F32)
        p2 = (q * q).astype(F32)
        p2h = p2.astype(bf)
        p2l = (p2 - p2h.astype(F32)).astype(F32).astype(bf)
        qb = q.astype(bf)
        one = np.ones_like(q, dtype=bf)
        zero = np.zeros_like(q, dtype=bf)
        return np.stack([p2h, p2l, p2h, p2l, p2h, p2l,
                         qb, qb, qb, one, one, one, zero, zero, zero, zero])

    # basis_y [64, 512]: 4-chunk groups; block-diag: rows 16*jin+r active in
    # cols 128*jin + py
    py = np.arange(128) - 64.0
    by_rows = basis_rows(py)                      # [16, 128]
    basis_y = np.zeros((64, 512), bf)
    for jin in range(4):
        basis_y[16 * jin:16 * jin + 16, 128 * jin:128 * jin + 128] = by_rows

    # basis_x per block b: [128, 256]: 8-chunk groups; cols 32*jin + px
    basis_xs = []
    for b in range(NBLK):
        px = np.arange(PXB * b, PXB * b + PXB) - 64.0
        bx_rows = basis_rows(px)                  # [16, 32]
        bx = np.zeros((128, 256), bf)
        for jin in range(8):
            bx[16 * jin:16 * jin + 16, 32 * jin:32 * jin + 32] = bx_rows
        basis_xs.append(bx)

    ident = np.eye(128, dtype=F32)

    in_maps = []
    for core in range(8):
        p, b = core // NBLK, core % NBLK
        inp128 = np.concatenate(
            [pos4, colors1, opac, ident, np.tile(rbs[p], (128, 1))],
            axis=1).astype(F32)                       # [128, 428]
        bas = np.zeros((128, 896), bf)
        bas[0:64, 0:512] = basis_y
        bas[64:128, 0:512] = basis_y
        bas[:, 512:768] = basis_xs[b]
        bas[:, 768:896] = np.eye(128, dtype=bf)
        in_maps.append({"inp128": inp128, "bas": bas})
    return in_maps


def _assemble(slabs):
    """slabs: list of 8 [128, 96] arrays -> [NPOSE*16, 3, 32, 32] output."""
    out = []
    for p in range(NPOSE):
        img = np.zeros((H, W, 3), F32)
        for b in range(NBLK):
            slab = slabs[p * NBLK + b]
            for c in range(3):
                img[:, PXB * b:PXB * b + PXB, c] = slab[:, 32 * c:32 * c + 32]
        tiles = img.reshape(H * W, 3).reshape(16, 1024, 3)
        tiles = tiles.transpose(0, 2, 1).reshape(16, 3, 32, 32)
        out.append(tiles)
    return np.concatenate(out, axis=0).astype(F32)


def kernel(positions, colors, opacities, scales, qvec, tvec, _trace=False):
    from concourse.bass_utils import run_bass_kernel_spmd

    if "nc" not in _CACHE:
        _CACHE["nc"] = _build_program()
    nc = _CACHE["nc"]

    in_maps = _host_prep(positions, colors, opacities, scales, qvec, tvec)
    res = run_bass_kernel_spmd(nc, in_maps, core_ids=list(range(8)),
                               trace=_trace)
    slabs = [np.asarray(res.results[c]["out"]) for c in range(8)]
    out = _assemble(slabs)
    if _trace:
        _CACHE["last_result"] = res
    return out
